# revision 1
# baseline (speedup 1.0000x reference)
"""Causal self-attention (B=4, T=2048, C=1024, H=16, D=64) on 8 TRN2 NeuronCores.

Sharding: core = 2*b + g  (b = batch 0..3, g = head-group 0..1; heads 8g..8g+7).
Each core computes, for its batch b and its 8 heads:
  qkv projection, causal softmax attention, and a PARTIAL output projection
  (its 512 rows of W_proj). Host sums the two partials per batch (+ b_proj).

Device-side layout (float32r matmuls: fp32 bits, PE rounds to ~12-bit
mantissa internally, 4x faster than plain fp32 matmul):
  - x arrives pre-transposed from host: xt [C=1024, T=2048]  (K-major)
  - q,k computed transposed:  qt/kt [128 (= 2 heads x 64 dims), T] per pair
  - v computed normal, with a ones column per head: V [T-tile 128, 8 x 65]
  - scores computed as S^T = K @ q^T : [128 keys, q-cols] in PSUM per key-tile
  - softmax without max-subtraction (scores ~ N(0,1); exp safe in fp32):
      P^T = exp(S^T/8) on ACT; causal diag block zeroed by GPSIMD affine_select
  - AV: lhsT = [V_h | 1] (M=65) -> psum rows 0:64 = Y^T, row 64 = denominator;
    normalize via DVE reciprocal + GPSIMD partition_broadcast + DVE multiply
  - out[q-tile, :] = sum_pairs ytT.T @ W_proj_rows  (q/k biases folded into
    the psum->SBUF copy as per-partition tensor_scalar adds, v bias via a
    rank-1 matmul; b_proj added on host during unshard)
"""

import sys

try:
    import concourse  # noqa: F401
except ImportError:
    sys.path.insert(0, "/opt/trn_rl_repo")

import numpy as np

import concourse.bacc as bacc
import concourse.mybir as mybir
import concourse.tile as tile

F32 = mybir.dt.float32
F32R = mybir.dt.float32r
AF = mybir.ActivationFunctionType

B, T, C = 4, 2048, 1024
H, D = 16, 64
NCORES = 8
HL = 8          # heads per core (local)
NPAIR = 4       # head pairs per core
CH = 1024       # q chunk (PSUM-sized)
NCH = T // CH   # 2
KT = T // 128   # 16 key tiles
CT = C // 128   # 8 contraction tiles over C
SCALE = 1.0 / 8.0  # 1/sqrt(D)

_prog_cache = {}


def build_program(debug=False):
    key = debug
    if key in _prog_cache:
        return _prog_cache[key]

    nc = bacc.Bacc(None, target_bir_lowering=False, debug=debug)

    xt = nc.dram_tensor("xt", [C, T], F32R, kind="ExternalInput")
    wq = nc.dram_tensor("wq", [C, 512], F32R, kind="ExternalInput")
    wk = nc.dram_tensor("wk", [C, 512], F32R, kind="ExternalInput")
    wv = nc.dram_tensor("wv", [C, 512], F32R, kind="ExternalInput")
    bqk_t = nc.dram_tensor("bqk_t", [128, 8], F32, kind="ExternalInput")
    bv = nc.dram_tensor("bv", [1, 512], F32R, kind="ExternalInput")
    wp = nc.dram_tensor("wp", [512, C], F32R, kind="ExternalInput")
    out = nc.dram_tensor("out", [T, C], F32, kind="ExternalOutput")

    with tile.TileContext(nc) as tc:
        with (
            tc.tile_pool(name="consts", bufs=1) as consts,
            tc.tile_pool(name="vp", bufs=1) as vp,
            tc.tile_pool(name="ytp", bufs=1) as ytp,
            tc.tile_pool(name="ptp", bufs=6) as ptp,
            tc.tile_pool(name="ps", bufs=2, space="PSUM") as ps,
        ):
            # ---- constants
            ones_stage = consts.tile([128, 512], F32, tag="ones_stage")
            nc.vector.memset(ones_stage, 1.0)
            triu_sb = consts.tile([128, 128], F32R, tag="triu")
            nc.gpsimd.memset(triu_sb.bitcast(mybir.dt.float32), 1.0)
            nc.gpsimd.affine_select(
                out=triu_sb.bitcast(mybir.dt.float32),
                in_=triu_sb.bitcast(mybir.dt.float32),
                compare_op=mybir.AluOpType.is_ge,
                fill=0.0, base=0, pattern=[[1, 128]], channel_multiplier=-1,
            )
            # rank-1 v-bias vectors; lhsT and rhs must share a base
            # partition (legal bases {0, 32, 64}): row 32 = bv(512)|ones(128)
            vecs = consts.tile([128, 640], F32R, tag="vecs")
            bv_sb = vecs[32:33, 0:512]
            nc.sync.dma_start(out=bv_sb, in_=bv[:, :])
            ones32_sb = vecs[32:33, 512:640]
            nc.vector.tensor_copy(ones32_sb, ones_stage[0:1, 0:128])
            # q/k bias as per-partition columns, folded into the psum->SBUF
            # copy of the transposed qk projection (tensor_scalar add)
            bqk_sb = consts.tile([128, 8], F32, tag="bqk")
            nc.sync.dma_start(out=bqk_sb, in_=bqk_t[:, :])

            yt_sb = []
            v_sb = []

            with tc.tile_pool(name="xtp", bufs=1) as xtp:
                # ---- resident xt [C, T] as 8 tiles of [128, T], loaded in
                # 512-col chunks so the V projection can start after ~1/4 of
                # the transfer instead of waiting for the full 8 MB.
                xt_r = xt.ap().rearrange("(k p) t -> k p t", p=128)
                xt_sb = [xtp.tile([128, T], F32R, tag=f"xt{k}", name=f"xt{k}")
                         for k in range(CT)]

                # ---- V projection: per t-tile [128, 8 heads x (64 V + 1 one)]
                with tc.tile_pool(name="wvp", bufs=1) as wvp:
                    wv_r = wv.ap().rearrange("(k p) n -> k p n", p=128)
                    wv_sb = []
                    for k in range(CT):
                        t_ = wvp.tile([128, 512], F32R, tag=f"wv{k}")
                        nc.sync.dma_start(out=t_, in_=wv_r[k])
                        wv_sb.append(t_)
                    # small 128-col prefix first: V t-tile 0 only needs
                    # xt[:, 0:128], so the first matmul starts ~3us earlier
                    for (c0_, c1_) in ((0, 128), (128, 512), (512, 1024),
                                       (1024, 1536), (1536, 2048)):
                        for k in range(CT):
                            nc.sync.dma_start(
                                out=xt_sb[k][:, c0_:c1_],
                                in_=xt_r[k][:, c0_:c1_],
                            )

                    for t in range(KT):
                        pv = ps.tile([128, 512], F32, tag="pqps", bufs=2)
                        for k in range(CT):
                            nc.tensor.matmul(
                                pv, lhsT=xt_sb[k][:, t * 128:(t + 1) * 128],
                                rhs=wv_sb[k], start=(k == 0), stop=False,
                            )
                        nc.tensor.matmul(
                            pv, lhsT=ones32_sb[0:1, 0:128], rhs=bv_sb,
                            start=False, stop=True,
                        )
                        vt = vp.tile([128, 520], F32R, tag=f"v{t}")
                        vt_r = vt.rearrange("p (h d) -> p h d", h=HL)
                        nc.vector.tensor_copy(vt_r[:, :, 0:64], pv)
                        nc.vector.tensor_copy(vt_r[:, :, 64:65], ones_stage[:, 0:HL])
                        v_sb.append(vt)

                # ---- head-pair loop: qk projection + attention
                with (
                    tc.tile_pool(name="qkt", bufs=2) as qkt,
                    tc.tile_pool(name="wqk", bufs=4) as wqk,
                ):
                    for p in range(NPAIR):
                        # qkT projection: qt/kt [128 (2 heads x 64 dims), T]
                        qt = qkt.tile([128, T], F32R, tag="qt")
                        kt = qkt.tile([128, T], F32R, tag="kt")
                        for side, (dst, wsrc) in enumerate(((qt, wq), (kt, wk))):
                            w8 = wqk.tile([128, CT, 128], F32R, tag="w")
                            w_src = wsrc.ap().rearrange(
                                "(k pp) m -> pp k m", pp=128
                            )[:, :, p * 128:(p + 1) * 128]
                            nc.sync.dma_start(out=w8, in_=w_src)
                            bcol = bqk_sb[:, 4 * side + p:4 * side + p + 1]
                            for nch in range(4):  # t chunks of 512
                                pq = ps.tile([128, 512], F32, tag="pqps", bufs=2)
                                for k in range(CT):
                                    nc.tensor.matmul(
                                        pq, lhsT=w8[:, k, :],
                                        rhs=xt_sb[k][:, nch * 512:(nch + 1) * 512],
                                        start=(k == 0), stop=(k == CT - 1),
                                    )
                                nc.vector.tensor_scalar(
                                    out=dst[:, nch * 512:(nch + 1) * 512],
                                    in0=pq, scalar1=bcol, scalar2=None,
                                    op0=mybir.AluOpType.add,
                                )

                        # attention for the two heads of this pair
                        yt = ytp.tile([128, T], F32R, tag=f"yt{p}")
                        yt_sb.append(yt)
                        for c in range(NCH):
                            for hh in range(2):
                                hloc = 2 * p + hh
                                base = 64 * hh
                                kmax = 8 * (c + 1)
                                b0_last = min(kmax - 1, 8 * c + 3)
                                ytps = ps.tile([65, CH], F32, tag="ytps", bufs=1,
                                               name=f"ytps{hloc}_{c}")
                                for ki in range(kmax):
                                    q_off = max(0, 128 * ki - CH * c)
                                    segs = []
                                    if q_off < 512:
                                        segs.append((q_off, 512))
                                    segs.append((max(q_off, 512), CH))
                                    stp = ps.tile([128, CH], F32, tag="stps", bufs=2,
                                                  name=f"stp{hloc}_{c}_{ki}")
                                    for (s0, s1) in segs:
                                        nc.tensor.matmul(
                                            stp[:, s0:s1],
                                            lhsT=kt[base:base + 64,
                                                    ki * 128:(ki + 1) * 128],
                                            rhs=qt[base:base + 64,
                                                   CH * c + s0:CH * c + s1],
                                            start=True, stop=True,
                                        )
                                    pt = ptp.tile([128, CH], F32R, tag="pt",
                                                  name=f"pt{hloc}_{c}_{ki}")
                                    nc.scalar.activation(
                                        out=pt[:, q_off:CH], in_=stp[:, q_off:CH],
                                        func=AF.Exp, scale=SCALE,
                                    )
                                    if ki >= 8 * c:  # causal mask on diag block
                                        nc.vector.tensor_mul(
                                            pt[:, q_off:q_off + 128],
                                            pt[:, q_off:q_off + 128], triu_sb,
                                        )
                                    for (s0, s1) in segs:
                                        last = b0_last if s0 < 512 else kmax - 1
                                        nc.tensor.matmul(
                                            ytps[0:65, s0:s1],
                                            lhsT=v_sb[ki][:, 65 * hloc:65 * hloc + 65],
                                            rhs=pt[:, s0:s1],
                                            start=(ki == 0), stop=(ki == last),
                                        )
                                rcp1 = ptp.tile([1, CH], F32, tag="pt",
                                                name=f"rcp{hloc}_{c}")
                                nc.vector.reciprocal(out=rcp1, in_=ytps[64:65, :])
                                rb = ptp.tile([64, CH], F32, tag="pt",
                                              name=f"rb{hloc}_{c}")
                                nc.gpsimd.partition_broadcast(rb, rcp1)
                                nc.vector.tensor_mul(
                                    yt[base:base + 64, CH * c:CH * (c + 1)],
                                    ytps[0:64, :], rb,
                                )

            # ---- output projection: out[qtile, :] = sum_p ytT.T @ wp_rows
            with (
                tc.tile_pool(name="wpp", bufs=1) as wpp,
                tc.tile_pool(name="outp", bufs=3) as outp,
            ):
                wp_r = wp.ap().rearrange("(k p) n -> k p n", p=128)
                wp_sb = []
                for k in range(NPAIR):
                    t_ = wpp.tile([128, C], F32R, tag=f"wp{k}")
                    nc.sync.dma_start(out=t_, in_=wp_r[k])
                    wp_sb.append(t_)

                for qt_i in range(KT):
                    pso = [
                        ps.tile([128, 512], F32, tag="pqps", bufs=2, name=f"pso{qt_i}_0"),
                        ps.tile([128, 512], F32, tag="pqps", bufs=2, name=f"pso{qt_i}_1"),
                    ]
                    for pr in range(NPAIR):
                        for nch in range(2):
                            nc.tensor.matmul(
                                pso[nch],
                                lhsT=yt_sb[pr][:, qt_i * 128:(qt_i + 1) * 128],
                                rhs=wp_sb[pr][:, nch * 512:(nch + 1) * 512],
                                start=(pr == 0), stop=(pr == NPAIR - 1),
                            )
                    ot = outp.tile([128, C], F32, tag="ot")
                    for nch in range(2):
                        nc.vector.tensor_copy(
                            ot[:, nch * 512:(nch + 1) * 512], pso[nch]
                        )
                    nc.sync.dma_start(
                        out=out.ap()[qt_i * 128:(qt_i + 1) * 128, :], in_=ot
                    )

    nc.compile()
    _prog_cache[key] = nc
    return nc


def shard_inputs(x, W_qkv, b_qkv, W_proj, core):
    b, g = core // 2, core % 2
    cq = slice(512 * g, 512 * g + 512)
    ck = slice(1024 + 512 * g, 1024 + 512 * g + 512)
    cv = slice(2048 + 512 * g, 2048 + 512 * g + 512)
    return {
        "xt": np.ascontiguousarray(x[b].T),
        "wq": np.ascontiguousarray(W_qkv[:, cq]),
        "wk": np.ascontiguousarray(W_qkv[:, ck]),
        "wv": np.ascontiguousarray(W_qkv[:, cv]),
        "bqk_t": np.stack(
            [b_qkv[cq].reshape(4, 128)[p_] for p_ in range(4)]
            + [b_qkv[ck].reshape(4, 128)[p_] for p_ in range(4)], axis=1
        ).astype(np.float32).copy(),
        "bv": np.ascontiguousarray(b_qkv[cv]).reshape(1, 512),
        "wp": np.ascontiguousarray(W_proj[512 * g:512 * g + 512, :]),
    }


def kernel(x, W_qkv, b_qkv, W_proj, b_proj, **run_kwargs):
    x = np.asarray(x, np.float32)
    W_qkv = np.asarray(W_qkv, np.float32)
    b_qkv = np.asarray(b_qkv, np.float32)
    W_proj = np.asarray(W_proj, np.float32)
    b_proj = np.asarray(b_proj, np.float32)

    nc = build_program()
    in_maps = [
        shard_inputs(x, W_qkv, b_qkv, W_proj, core) for core in range(NCORES)
    ]
    from concourse.bass_utils import run_bass_kernel_spmd

    res = run_bass_kernel_spmd(nc, in_maps, core_ids=list(range(NCORES)), **run_kwargs)
    outs = [r["out"] for r in res.results]
    full = np.stack([outs[2 * b_] + outs[2 * b_ + 1] + b_proj for b_ in range(B)])
    kernel.last_results = res
    return full

